# revision 19
# baseline (speedup 1.0000x reference)
"""Block-3D attention kernel for 8 Trainium2 NeuronCores.

Problem: B=2, 16x16x16 token grid, 8x8x8 blocks -> 16 independent blocks
of T=512 tokens. GQA attention (32 q heads, 8 kv heads, d=64) inside each
block, with QKV/O projections (hidden=2048).

Sharding: pure data-parallel over blocks - 2 blocks per core, full
weights replicated, no collectives. Each core runs an identical program
on its own slice.

Schedule (all matmuls bf16, fp32 PSUM; measured ~411us vs 430us baseline):
  - hbT/wv loads interleaved across two DMA queues (sync + gpsimd); V
    projection is k-outer across all 8 PSUM banks so the PE chases the
    DMA from ~12us (the first ~9.5MB of loads are HBM-bandwidth bound);
    the last two k-chunks interleave per-tile with their drain casts,
    alternated between DVE and ACT, so K proj isn't gated on a cast tail.
  - attention pipelined: each (head-pair, block) unit's QK scores go
    through [128,1024] PSUM tiles (2 row-tiled concurrent 64-contraction
    matmuls each, ping-pong bufs=2) and the previous unit's PV matmuls
    are interleaved between QK units so the PE never waits on ACT exp.
  - softmax denominators ride the PV matmul for free (ones columns in the
    v tiles -> psum rows 64-127); their reciprocal is computed per unit as
    exp(-ln(x)) on ACT -- ln and exp share one activation table
    (natural_log_exp_and_others), so normalization costs zero table swaps
    and spreads evenly between the score exps instead of bulging at group
    boundaries (the reciprocal-table round-trip used to cost ~9us/group
    and tripped the HAM clock gate); the normalize multiplies run one
    unit behind on DVE so its FIFO never blocks on the ACT chain.
  - o^T accumulates into per-group tiles; the output projection's first
    tile runs contraction chunks 0-11 (groups 0-2) for both blocks before
    touching group 3, hiding the last group's normalization; Wo tiles are
    triple-buffer prefetched on the gpsimd queue.
"""

import numpy as np
import ml_dtypes

import concourse.bass as bass
import concourse.mybir as mybir
from concourse.tile import TileContext
from concourse.bass_utils import run_bass_kernel_spmd

# ---------------------------------------------------------------------------
# Workaround for this walrus build: at most 1 sync wait per Drain
# instruction, but TileContext's tail drain collects one wait per active
# proc. Split the waits across per-proc NOPs on the sync engine.
# ---------------------------------------------------------------------------
from concourse import tile as _tile
from concourse.vector_clock import ScopedClock as _ScopedClock
from concourse.vector_clock import VectorClock as _VectorClock
from concourse.tile_sem_assignment import N_PROCS as _N_PROCS


def _split_drain_and_barrier(self, tick_clock, wait_clock):
    gc = tick_clock.global_clock
    for p in range(_N_PROCS):
        if gc[p] == 0:
            continue
        c = _VectorClock([gc[q] if q == p else 0 for q in range(_N_PROCS)])
        nop = self.nc.sync.nop(nofuse=True)
        wait_clock.add_sem_waits(nop.ins, _ScopedClock({None: c}))
    # The NOPs above precede the drain in SP program order and carry all
    # required waits, so the drain itself needs none.
    self.nc.sync.drain()
    self.nc.all_engine_barrier()
    assert self.sems is not None
    popped = self.nc._tile_sem_poison_stack.pop()
    assert popped is self._sem_poison
    self.nc.clear_and_free_semaphores(list(self.sems.allocated().values()))
    self.nc.all_engine_barrier()


_tile.TileContext._drain_and_barrier = _split_drain_and_barrier

# This walrus also caps sync waits per regular instruction (observed: 3
# waits on a DVE TensorCopy rejected). Post-pass: move excess waits onto
# bass_nofuse NOPs inserted immediately before the instruction on the
# same engine.
_WAIT_CAP = 1

from concourse.tile_rust import add_dep_helper as _add_dep_helper


def _add_dep(from_inst, to_inst, reason=""):
    _add_dep_helper(from_inst, to_inst, sync=False, reason=reason)


def _act_reciprocal(nc, out, in_):
    """Reciprocal on the Scalar (ACT) engine. bass blocks
    ActivationFunctionType.Reciprocal for accuracy; measured on this HW the
    rel err is ~1.2e-5 for inputs in [300, 2500] (our softmax denominators),
    far below this kernel's bf16-dominated error floor, and it is ~5x
    cheaper than the exact DVE reciprocal at free size 512."""
    eng = nc.scalar
    return eng.add_instruction(
        mybir.InstActivation(
            name=nc.get_next_instruction_name(),
            func=mybir.ActivationFunctionType.Reciprocal,
            ins=[eng.lower_ap(in_),
                 mybir.ImmediateValue(dtype=mybir.dt.float32, value=0.0),
                 mybir.ImmediateValue(dtype=mybir.dt.float32, value=1.0),
                 mybir.ImmediateValue(dtype=mybir.dt.float32, value=0.0)],
            outs=[eng.lower_ap(out)],
        )
    )


def _split_excess_waits(nc, cap=_WAIT_CAP):
    count = 0
    for f in nc.m.functions:
        for bb in f.blocks:
            il = bb.instructions
            i = 0
            while i < len(il):
                inst = il[i]
                si = inst.sync_info
                c = 1 if isinstance(inst, mybir.InstDrain) else cap
                if si is not None and len(si.on_wait) > c:
                    waits = list(si.on_wait)
                    keep = waits[-c:] if c else []
                    excess = waits[:-c] if c else waits
                    pos = i
                    for g0 in range(0, len(excess), cap):
                        grp = excess[g0:g0 + cap]
                        count += 1
                        nop = mybir.InstNoOp(
                            name=f"waitsplit_{count}",
                            sync_info=mybir.SyncInfo(on_wait=grp, on_update=[]),
                            bass_nofuse=True,
                            engine=inst.engine,
                        )
                        il.insert(pos, nop)
                        pos += 1
                        i += 1
                    si.on_wait = keep
                i += 1
    return count

def _elide_redundant_updates(nc):
    """Per-instruction semaphore increments serialize on the engine's
    EVT_SEM write port (~26ns each). For any semaphore whose updates are
    all +1 increments from a single engine (so completion order == program
    order) and whose waits are all >=-immediate, only the updates that are
    the exact target of some wait are observable: a wait with threshold T
    is satisfied precisely when the T-th update lands. Keep those targets,
    drop the rest, and renumber every wait's threshold."""
    fns = nc.m.functions
    upd_by_sem = {}
    wait_refs_by_sem = {}
    for f in fns:
        for bb in f.blocks:
            for inst in bb.instructions:
                si = inst.sync_info
                if not si:
                    continue
                for u in si.on_update:
                    upd_by_sem.setdefault(u.id, []).append((inst, u))
                for w in si.on_wait:
                    wait_refs_by_sem.setdefault(w.id, []).append(w)

    n_removed = 0
    for sid, updates in upd_by_sem.items():
        waits = wait_refs_by_sem.get(sid, [])
        if not all(u.update_mode == "sem-inc" and u.update_value == 1
                   for _, u in updates):
            continue
        engines = {inst.engine for inst, _ in updates}
        if len(engines) != 1:
            continue
        if not all(w.wait_mode == "sem-ge-imm" for w in waits):
            continue
        if any(w.wait_value > len(updates) or w.wait_value < 1 for w in waits):
            continue
        # 1-based target indices that must survive; always keep the final
        # update so the value a drain might observe still advances fully.
        targets = {w.wait_value for w in waits}
        targets.add(len(updates))
        new_rank = {}
        rank = 0
        for idx, (inst, u) in enumerate(updates, start=1):
            if idx in targets:
                rank += 1
                new_rank[idx] = rank
            else:
                inst.sync_info.on_update = [
                    x for x in inst.sync_info.on_update if x is not u
                ]
                n_removed += 1
        for w in waits:
            w.wait_value = new_rank[w.wait_value]
    return n_removed


# ---------------------------------------------------------------------------
# Model constants (hardcoded per problem spec)
# ---------------------------------------------------------------------------
HID = 2048
NH = 32
NKV = 8
D = 64
B = 2
GRID = 16           # x_dim = y_dim = z_dim
BS = 8              # block size per axis
T = BS * BS * BS    # 512 tokens per block
NBLOCKS = 16        # total 3D blocks (B * 2*2*2)
N_CORES = 8
BPC = NBLOCKS // N_CORES  # blocks per core = 2
TC = BPC * T        # tokens per core = 1024
KC = HID // 128     # 16 contraction chunks

BF16 = mybir.dt.bfloat16
F32 = mybir.dt.float32

_PROGRAM = None


def _build_program():
    nc = bass.Bass("TRN2", target_bir_lowering=False, debug=False,
                   num_devices=N_CORES)

    hbT = nc.dram_tensor("hbT", [HID, TC], BF16, kind="ExternalInput")
    wqT = nc.dram_tensor("wqT", [HID, NH * D], BF16, kind="ExternalInput")
    wkT = nc.dram_tensor("wkT", [HID, NKV * D], BF16, kind="ExternalInput")
    wvT = nc.dram_tensor("wvT", [HID, NKV * D], BF16, kind="ExternalInput")
    woT = nc.dram_tensor("woT", [NH * D, HID], BF16, kind="ExternalInput")
    out = nc.dram_tensor("out", [HID, TC], F32, kind="ExternalOutput")

    QW = NH * D       # 2048
    KW = NKV * D      # 512

    with TileContext(nc) as tc:
        with tc.tile_pool(name="persist", bufs=1) as cpool:
            # kTd per group g (kv heads 2g, 2g+1): local head jl on both
            # partition halves; cols jl*TC + b*T + t
            kTdg = [cpool.tile([128, 2 * TC], BF16, tag=f"kTd{g}",
                               name=f"kTd{g}")
                    for g in range(4)]
            # v_sbg[b][sc]: [s=128, j*128 + (v_j d | ones)] per (block,
            # s-chunk); ones cols make the PV matmul emit the softmax
            # denominator on psum rows 64-127.
            v_sbg = [[cpool.tile([128, NKV * 2 * D], BF16,
                                 tag=f"vsb{b}{sc}", name=f"vsb{b}{sc}")
                      for sc in range(4)] for b in range(2)]
            # ones-fill: only the odd 64-col blocks; on DVE (idle at start),
            # per-(b,sc) so V-proj casts pipeline behind them.
            for b in range(BPC):
                for sc in range(4):
                    dst = v_sbg[b][sc][:, :].rearrange(
                        "p (j e) -> p j e", e=2 * D)[:, :, D:2 * D]
                    nc.vector.memset(dst, 1.0)

            # o^T per (group, block): [128 = pair d, pl*T + t]
            oTbg = [[cpool.tile([128, 4 * T], BF16, tag=f"oT{g}{b}",
                                name=f"oT{g}{b}")
                     for b in range(BPC)] for g in range(4)]

            with (
                tc.tile_pool(name="wo", bufs=3) as wopool,
                tc.tile_pool(name="outsb", bufs=2) as outpool,
                tc.tile_pool(name="chunks", bufs=1) as ckpool,
            ):
                # hbT + wv chunk loads interleaved on two DMA queues so
                # chunk k lands ~k*1us in; V proj (k-outer) chases them.
                # chunk 0 is split per block so the first V matmul only
                # waits on wv0 + half of hb chunk 0
                hbk0 = [ckpool.tile([128, T], BF16, tag=f"hbk0{b}",
                                    name=f"hbk0{b}") for b in range(BPC)]
                hbk = [None] + [ckpool.tile([128, TC], BF16, tag=f"hbk{k}",
                                            name=f"hbk{k}")
                                for k in range(1, KC)]

                def hb(k, b, c0, c1):
                    if k == 0:
                        return hbk0[b][:, c0:c1]
                    return hbk[k][:, T * b + c0:T * b + c1]

                def load_wk():
                    # per-kv-head-group column strips: K proj for group g
                    # only waits on its own 0.5MB strip (group 0 lands
                    # right after the hb evens), and the sync queue issues
                    # 4 DMAs instead of 16
                    ts = []
                    for g in range(4):
                        t = ckpool.tile([128, KC * 128], BF16, tag=f"wkg{g}",
                                        name=f"wkg{g}")
                        nc.sync.dma_start(
                            out=t[:, :].rearrange("p (k m) -> p k m", m=128),
                            in_=wkT[:, 128 * g:128 * (g + 1)]
                            .rearrange("(k p) m -> p k m", p=128),
                        )
                        ts.append(t)
                    return ts

                def load_wq_quarter(q):
                    # alternating tags: quarter q's DMA waits only on
                    # quarter q-2's readers, so it prefetches one group
                    # ahead and overlaps the previous group's matmuls
                    ts = []
                    for k in range(KC):
                        t = ckpool.tile([128, QW // 4], BF16,
                                        tag=f"wq{'AB'[q % 2]}{k}",
                                        name=f"wq{q}_{k}")
                        nc.sync.dma_start(
                            out=t[:, :],
                            in_=wqT[128 * k:128 * (k + 1),
                                    (QW // 4) * q:(QW // 4) * (q + 1)])
                        ts.append(t)
                    return ts

                # ---------------- V projection, k-outer -------------------
                # 8 psum banks (b, c); matmuls for chunk k start as soon as
                # hbk[k]/wvk[k] land. Last chunk's matmuls interleave with
                # their psum-drain casts so K proj isn't gated on a serial
                # cast tail (the next PSUM pools reuse these banks).
                with (
                    tc.tile_pool(name="wvp", bufs=1) as wvpool,
                    tc.tile_pool(name="ps_v", bufs=1, space="PSUM") as ps_v,
                ):
                    wvk = [wvpool.tile([128, KW], BF16, tag=f"wvk{k}",
                                       name=f"wvk{k}") for k in range(KC)]
                    for k in range(KC):
                        q = nc.sync if k % 2 == 0 else nc.gpsimd
                        q.dma_start(out=wvk[k][:, :],
                                    in_=wvT[128 * k:128 * (k + 1), :])
                        if k == 0:
                            for b in range(BPC):
                                q.dma_start(
                                    out=hbk0[b][:, :],
                                    in_=hbT[0:128, T * b:T * (b + 1)])
                        else:
                            q.dma_start(out=hbk[k][:, :],
                                        in_=hbT[128 * k:128 * (k + 1), :])
                    vps = [[ps_v.tile([128, KW], F32, tag=f"psv{b}{c}", name=f"psv{b}{c}")
                            for c in range(4)] for b in range(BPC)]
                    for k in range(KC - 2):
                        for b in range(BPC):
                            for c in range(4):
                                nc.tensor.matmul(
                                    vps[b][c][:, :],
                                    lhsT=hb(k, b, 128 * c, 128 * c + 128),
                                    rhs=wvk[k][:, :],
                                    start=(k == 0), stop=False,
                                )
                    wkk = load_wk()
                    wqk = load_wq_quarter(0)
                    # last two chunks per tile, cast immediately after each
                    # tile's stop so the 8 drain casts overlap the remaining
                    # matmuls instead of trailing them
                    for b in range(BPC):
                        for c in range(4):
                            for k in (KC - 2, KC - 1):
                                nc.tensor.matmul(
                                    vps[b][c][:, :],
                                    lhsT=hb(k, b, 128 * c, 128 * c + 128),
                                    rhs=wvk[k][:, :],
                                    start=False, stop=(k == KC - 1),
                                )
                            dst = v_sbg[b][c][:, :].rearrange(
                                "p (j e) -> p j e", e=2 * D)[:, :, 0:D]
                            srcv = vps[b][c][:, :].rearrange(
                                "p (j d) -> p j d", d=D)
                            # alternate the 8 drain casts between DVE and
                            # ACT (copy is in every act table) so the next
                            # PSUM pools aren't gated on a serial cast tail
                            if c % 2 == 0:
                                nc.vector.tensor_copy(dst, srcv)
                            else:
                                nc.scalar.activation(
                                    dst, srcv,
                                    mybir.ActivationFunctionType.Copy)

                def load_wo(mc):
                    wo = wopool.tile([128, KC * 128], BF16, tag="wo", name="wo")
                    nc.gpsimd.dma_start(
                        out=wo[:, :].rearrange("p (k m) -> p k m", m=128),
                        in_=woT[:, 128 * mc:128 * (mc + 1)]
                        .rearrange("(k p) m -> p k m", p=128),
                    )
                    return wo

                wo_tiles = []

                with (
                    tc.tile_pool(name="qTp", bufs=4) as qpool,
                    tc.tile_pool(name="pT", bufs=12) as ppool,
                    tc.tile_pool(name="lv", bufs=4) as lvpool,
                    tc.tile_pool(name="ps_proj", bufs=2, space="PSUM") as ps_proj,
                    tc.tile_pool(name="ps_st", bufs=2, space="PSUM") as ps_st,
                    tc.tile_pool(name="ps_pv", bufs=2, space="PSUM") as ps_pv,
                ):
                    def k_proj(g):
                        for b in range(BPC):
                            ps = ps_proj.tile([128, T], F32, tag="ps", name="ps")
                            for k in range(KC):
                                nc.tensor.matmul(
                                    ps[:, :],
                                    lhsT=wkk[g][:, 128 * k:128 * k + 128],
                                    rhs=hb(k, b, 0, T),
                                    start=(k == 0), stop=(k == KC - 1),
                                )
                            for jl, lo in ((0, 0), (1, 64)):
                                src = ps[lo:lo + 64, :]
                                nc.vector.tensor_copy(
                                    kTdg[g][0:64,
                                            TC * jl + T * b:TC * jl + T * (b + 1)],
                                    src)
                                nc.vector.tensor_copy(
                                    kTdg[g][64:128,
                                            TC * jl + T * b:TC * jl + T * (b + 1)],
                                    src)

                    # --- pipelined attention unit machinery ---------------
                    # One "unit" = (pair, b). QK+exp of unit i interleave
                    # with PV of unit i-1.
                    class Unit:
                        def __init__(self, g, pl, b, qTp):
                            self.g, self.pl, self.b, self.qTp = g, pl, b, qTp
                            self.pts = []     # 4 exp'd score tiles
                            self.last_exp = None

                    def emit_qk_pair(u, sci2):
                        """Emit QK unit sci2 (s-chunk) for unit u: 2
                        row-tiled concurrent MMs + exp."""
                        g, pl, b = u.g, u.pl, u.b
                        jl = pl // 2
                        st = ps_st.tile([128, 2 * T], F32, tag="st", name="st")
                        for half in range(2):
                            nc.tensor.matmul(
                                st[:, T * half:T * (half + 1)],
                                lhsT=kTdg[g][64 * half:64 * half + 64,
                                             TC * jl + T * b + 128 * sci2:
                                             TC * jl + T * b + 128 * sci2 + 128],
                                rhs=u.qTp[64 * half:64 * half + 64, :],
                                start=True, stop=True,
                            )
                        p_t = ppool.tile([128, 2 * T], BF16, tag="pT", name="pT")
                        ei = nc.scalar.activation(
                            p_t[:, :], st[:, :],
                            mybir.ActivationFunctionType.Exp,
                        )
                        u.last_exp = ei.ins
                        u.pts.append(p_t)

                    def emit_pv_half(u, half, group_lvs):
                        g, pl, b = u.g, u.pl, u.b
                        jl = pl // 2
                        po = ps_pv.tile([128, T], F32, tag="po", name="po")
                        for sc in range(4):
                            nc.tensor.matmul(
                                po[:, :],
                                lhsT=v_sbg[b][sc][:, 128 * (2 * g + jl):
                                                  128 * (2 * g + jl) + 128],
                                rhs=u.pts[sc][:, T * half:T * (half + 1)],
                                start=(sc == 0), stop=(sc == 3),
                            )
                        u.po = getattr(u, 'po', [None, None])
                        u.po[half] = po
                        if half == 1:
                            # park denominators (f32) + unnormalized o^T;
                            # psum frees immediately. Reciprocal per unit as
                            # exp(-ln(x)) on ACT: ln and exp live in ONE act
                            # table (natural_log_exp_and_others), so this
                            # costs zero table swaps and spreads evenly
                            # between the score exps instead of bulging at
                            # group boundaries like the reciprocal table
                            # round-trip did.
                            lv = lvpool.tile([128, T], F32, tag="lv",
                                             name="lv")
                            nc.vector.tensor_copy(lv[0:64, :],
                                                  u.po[0][64:128, :])
                            nc.vector.tensor_copy(lv[64:128, :],
                                                  u.po[1][64:128, :])
                            nc.vector.tensor_copy(
                                oTbg[g][b][0:64, T * (pl % 4):T * (pl % 4 + 1)],
                                u.po[0][0:64, :])
                            nc.vector.tensor_copy(
                                oTbg[g][b][64:128, T * (pl % 4):T * (pl % 4 + 1)],
                                u.po[1][0:64, :])
                            nc.scalar.activation(
                                lv[:, :], lv[:, :],
                                mybir.ActivationFunctionType.Ln)
                            nc.scalar.activation(
                                lv[:, :], lv[:, :],
                                mybir.ActivationFunctionType.Exp,
                                scale=-1.0)
                            pending_tt.append((u.g, u.pl, u.b, lv))

                    def drain_one_tt():
                        # normalize multiplies run one unit behind so the
                        # DVE never blocks waiting on this unit's ACT chain
                        if not pending_tt:
                            return
                        g, pl, b, lv = pending_tt.pop(0)
                        nc.vector.tensor_tensor(
                            out=oTbg[g][b][:, T * (pl % 4):T * (pl % 4 + 1)],
                            in0=oTbg[g][b][:, T * (pl % 4):T * (pl % 4 + 1)],
                            in1=lv[:, :],
                            op=mybir.AluOpType.mult,
                        )

                    pending = []          # units whose PV is not yet emitted
                    pending_tt = []       # units normalized but not yet multiplied

                    def drain_pending_half(_unused):
                        """Emit the next PV half of the oldest pending
                        unit."""
                        if not pending:
                            return
                        u = pending[0]
                        h = getattr(u, 'next_half', 0)
                        emit_pv_half(u, h, None)
                        if h == 1:
                            pending.pop(0)
                            drain_one_tt()
                        else:
                            u.next_half = 1

                    glvs = [[] for _ in range(4)]
                    for g in range(4):
                        k_proj(g)
                        if g < 3:
                            wqk_next = load_wq_quarter(g + 1)
                        for i, pl in enumerate(range(4)):
                            mq = 4 * g + pl
                            units = []
                            for b in range(BPC):
                                qTp = qpool.tile([128, T], BF16, tag="qTp", name="qTp")
                                ps = ps_proj.tile([128, T], F32, tag="ps", name="ps")
                                for k in range(KC):
                                    nc.tensor.matmul(
                                        ps[:, :],
                                        lhsT=wqk[k][:, 128 * pl:128 * pl + 128],
                                        rhs=hb(k, b, 0, T),
                                        start=(k == 0), stop=(k == KC - 1),
                                    )
                                nc.vector.tensor_copy(qTp[:, :], ps[:, :])
                                units.append(Unit(g, pl, b, qTp))
                            for u in units:
                                emit_qk_pair(u, 0)
                                emit_qk_pair(u, 1)
                                drain_pending_half(glvs)
                                emit_qk_pair(u, 2)
                                emit_qk_pair(u, 3)
                                drain_pending_half(glvs)
                                pending.append(u)
                        if g < 3:
                            wqk = wqk_next

                    # drain the tail of the PV pipeline and the last
                    # normalization multiplies
                    while pending:
                        drain_pending_half(glvs)
                        drain_pending_half(glvs)
                    while pending_tt:
                        drain_one_tt()

                # ------------ output projection ----------------------
                wo_tiles.extend(load_wo(mc) for mc in range(3))
                with tc.tile_pool(name="ps_wo", bufs=2, space="PSUM") as ps_wo:
                    for mc in range(KC):
                        wo = wo_tiles[mc]
                        if mc + 3 < KC:
                            wo_tiles.append(load_wo(mc + 3))
                        if mc == 0:
                            # first tile: run groups 0-2 of both blocks
                            # before touching group 3 (whose normalize may
                            # still be in flight on DVE).
                            pss = []
                            for b in range(BPC):
                                ps = ps_wo.tile([128, T], F32, tag="psf", name="psf")
                                pss.append(ps)
                                for k in range(12):
                                    nc.tensor.matmul(
                                        ps[:, :],
                                        lhsT=wo[:, 128 * k:128 * k + 128],
                                        rhs=oTbg[k // 4][b][:, T * (k % 4):
                                                            T * (k % 4 + 1)],
                                        start=(k == 0), stop=False,
                                    )
                            for b in range(BPC):
                                ps = pss[b]
                                for k in range(12, KC):
                                    nc.tensor.matmul(
                                        ps[:, :],
                                        lhsT=wo[:, 128 * k:128 * k + 128],
                                        rhs=oTbg[k // 4][b][:, T * (k % 4):
                                                            T * (k % 4 + 1)],
                                        start=False, stop=(k == KC - 1),
                                    )
                                osb = outpool.tile([128, T], F32, tag="osb", name="osb")
                                nc.vector.tensor_copy(osb[:, :], ps[:, :])
                                nc.sync.dma_start(
                                    out=out[128 * mc:128 * (mc + 1),
                                            T * b:T * (b + 1)],
                                    in_=osb[:, :],
                                )
                        else:
                            for b in range(BPC):
                                ps = ps_wo.tile([128, T], F32, tag="psf", name="psf")
                                for k in range(KC):
                                    nc.tensor.matmul(
                                        ps[:, :],
                                        lhsT=wo[:, 128 * k:128 * k + 128],
                                        rhs=oTbg[k // 4][b][:, T * (k % 4):
                                                            T * (k % 4 + 1)],
                                        start=(k == 0), stop=(k == KC - 1),
                                    )
                                osb = outpool.tile([128, T], F32, tag="osb", name="osb")
                                nc.vector.tensor_copy(osb[:, :], ps[:, :])
                                nc.sync.dma_start(
                                    out=out[128 * mc:128 * (mc + 1),
                                            T * b:T * (b + 1)],
                                    in_=osb[:, :],
                                )

    _split_excess_waits(nc)
    return nc


def _get_program():
    global _PROGRAM
    if _PROGRAM is None:
        _PROGRAM = _build_program()
    return _PROGRAM


def _to_blocks_tokens(x):
    """[B, L, F] -> [NBLOCKS, T, F] with the reference's 3D block order."""
    Bn, L, F = x.shape
    n = GRID // BS
    x = x.reshape(Bn, n, BS, n, BS, n, BS, F)
    x = x.transpose(0, 1, 3, 5, 2, 4, 6, 7)
    return x.reshape(Bn * n * n * n, BS * BS * BS, F)


def _from_blocks_tokens(x):
    """[NBLOCKS, T, F] -> [B, L, F] inverse of _to_blocks_tokens."""
    NBf, Tf, F = x.shape
    n = GRID // BS
    x = x.reshape(B, n, n, n, BS, BS, BS, F)
    x = x.transpose(0, 1, 4, 2, 5, 3, 6, 7)
    return x.reshape(B, GRID * GRID * GRID, F)


def kernel(hidden_states, Wq, Wk, Wv, Wo, x_dim, y_dim, z_dim):
    hidden_states = np.asarray(hidden_states, dtype=np.float32)
    Wq = np.asarray(Wq, dtype=np.float32)
    Wk = np.asarray(Wk, dtype=np.float32)
    Wv = np.asarray(Wv, dtype=np.float32)
    Wo = np.asarray(Wo, dtype=np.float32)

    bf = ml_dtypes.bfloat16
    scale = 1.0 / np.sqrt(D)
    wqT = np.ascontiguousarray((Wq.T * scale).astype(bf))  # [HID, 2048]
    wkT = np.ascontiguousarray(Wk.T.astype(bf))            # [HID, 512]
    wvT = np.ascontiguousarray(Wv.T.astype(bf))            # [HID, 512]
    woT = np.ascontiguousarray(Wo.T.astype(bf))            # [2048, HID]

    blocks = _to_blocks_tokens(hidden_states)              # [16, 512, HID]

    in_maps = []
    for c in range(N_CORES):
        hb = blocks[BPC * c:BPC * (c + 1)]                 # [2, 512, HID]
        hbT = np.ascontiguousarray(
            hb.transpose(2, 0, 1).reshape(HID, TC).astype(bf)
        )
        in_maps.append({
            "hbT": hbT, "wqT": wqT, "wkT": wkT, "wvT": wvT, "woT": woT,
        })

    global _LAST_IN_MAPS
    _LAST_IN_MAPS = in_maps
    nc = _get_program()
    res = run_bass_kernel_spmd(nc, in_maps, list(range(N_CORES)))

    out_blocks = np.empty((NBLOCKS, T, HID), dtype=np.float32)
    for c in range(N_CORES):
        o = res.results[c]["out"]                          # [HID, 1024]
        for b in range(BPC):
            out_blocks[BPC * c + b] = o[:, T * b:T * (b + 1)].T
    return _from_blocks_tokens(out_blocks)


# revision 20
# speedup vs baseline: 1.1813x; 1.1813x over previous
"""Block-3D attention kernel for 8 Trainium2 NeuronCores.

Problem: B=2, 16x16x16 token grid, 8x8x8 blocks -> 16 independent blocks
of T=512 tokens. GQA attention (32 q heads, 8 kv heads, d=64) inside each
block, with QKV/O projections (hidden=2048).

Sharding: pure data-parallel over blocks - 2 blocks per core, full
weights replicated, no collectives. Each core runs an identical program
on its own slice.

Schedule (all matmuls bf16, fp32 PSUM; measured ~411us vs 430us baseline):
  - hbT/wv loads interleaved across two DMA queues (sync + gpsimd); V
    projection is k-outer across all 8 PSUM banks so the PE chases the
    DMA from ~12us (the first ~9.5MB of loads are HBM-bandwidth bound);
    the last two k-chunks interleave per-tile with their drain casts,
    alternated between DVE and ACT, so K proj isn't gated on a cast tail.
  - attention pipelined: each (head-pair, block) unit's QK scores go
    through [128,1024] PSUM tiles (2 row-tiled concurrent 64-contraction
    matmuls each, ping-pong bufs=2) and the previous unit's PV matmuls
    are interleaved between QK units so the PE never waits on ACT exp.
  - softmax denominators ride the PV matmul for free (ones columns in the
    v tiles -> psum rows 64-127); their reciprocal is computed per unit as
    exp(-ln(x)) on ACT -- ln and exp share one activation table
    (natural_log_exp_and_others), so normalization costs zero table swaps
    and spreads evenly between the score exps instead of bulging at group
    boundaries (the reciprocal-table round-trip used to cost ~9us/group
    and tripped the HAM clock gate); the normalize multiplies run one
    unit behind on DVE so its FIFO never blocks on the ACT chain.
  - o^T accumulates into per-group tiles; the output projection's first
    tile runs contraction chunks 0-11 (groups 0-2) for both blocks before
    touching group 3, hiding the last group's normalization; Wo tiles are
    triple-buffer prefetched on the gpsimd queue.
"""

import numpy as np
import ml_dtypes

import concourse.bass as bass
import concourse.mybir as mybir
from concourse.tile import TileContext
from concourse.bass_utils import run_bass_kernel_spmd

# ---------------------------------------------------------------------------
# Workaround for this walrus build: at most 1 sync wait per Drain
# instruction, but TileContext's tail drain collects one wait per active
# proc. Split the waits across per-proc NOPs on the sync engine.
# ---------------------------------------------------------------------------
from concourse import tile as _tile
from concourse.vector_clock import ScopedClock as _ScopedClock
from concourse.vector_clock import VectorClock as _VectorClock
from concourse.tile_sem_assignment import N_PROCS as _N_PROCS


def _split_drain_and_barrier(self, tick_clock, wait_clock):
    gc = tick_clock.global_clock
    for p in range(_N_PROCS):
        if gc[p] == 0:
            continue
        c = _VectorClock([gc[q] if q == p else 0 for q in range(_N_PROCS)])
        nop = self.nc.sync.nop(nofuse=True)
        wait_clock.add_sem_waits(nop.ins, _ScopedClock({None: c}))
    # The NOPs above precede the drain in SP program order and carry all
    # required waits, so the drain itself needs none.
    self.nc.sync.drain()
    self.nc.all_engine_barrier()
    assert self.sems is not None
    popped = self.nc._tile_sem_poison_stack.pop()
    assert popped is self._sem_poison
    self.nc.clear_and_free_semaphores(list(self.sems.allocated().values()))
    self.nc.all_engine_barrier()


_tile.TileContext._drain_and_barrier = _split_drain_and_barrier

# This walrus also caps sync waits per regular instruction (observed: 3
# waits on a DVE TensorCopy rejected). Post-pass: move excess waits onto
# bass_nofuse NOPs inserted immediately before the instruction on the
# same engine.
_WAIT_CAP = 1

from concourse.tile_rust import add_dep_helper as _add_dep_helper


def _add_dep(from_inst, to_inst, reason=""):
    _add_dep_helper(from_inst, to_inst, sync=False, reason=reason)


def _act_reciprocal(nc, out, in_):
    """Reciprocal on the Scalar (ACT) engine. bass blocks
    ActivationFunctionType.Reciprocal for accuracy; measured on this HW the
    rel err is ~1.2e-5 for inputs in [300, 2500] (our softmax denominators),
    far below this kernel's bf16-dominated error floor, and it is ~5x
    cheaper than the exact DVE reciprocal at free size 512."""
    eng = nc.scalar
    return eng.add_instruction(
        mybir.InstActivation(
            name=nc.get_next_instruction_name(),
            func=mybir.ActivationFunctionType.Reciprocal,
            ins=[eng.lower_ap(in_),
                 mybir.ImmediateValue(dtype=mybir.dt.float32, value=0.0),
                 mybir.ImmediateValue(dtype=mybir.dt.float32, value=1.0),
                 mybir.ImmediateValue(dtype=mybir.dt.float32, value=0.0)],
            outs=[eng.lower_ap(out)],
        )
    )


def _split_excess_waits(nc, cap=_WAIT_CAP):
    count = 0
    for f in nc.m.functions:
        for bb in f.blocks:
            il = bb.instructions
            i = 0
            while i < len(il):
                inst = il[i]
                si = inst.sync_info
                c = 1 if isinstance(inst, mybir.InstDrain) else cap
                if si is not None and len(si.on_wait) > c:
                    waits = list(si.on_wait)
                    keep = waits[-c:] if c else []
                    excess = waits[:-c] if c else waits
                    pos = i
                    for g0 in range(0, len(excess), cap):
                        grp = excess[g0:g0 + cap]
                        count += 1
                        nop = mybir.InstNoOp(
                            name=f"waitsplit_{count}",
                            sync_info=mybir.SyncInfo(on_wait=grp, on_update=[]),
                            bass_nofuse=True,
                            engine=inst.engine,
                        )
                        il.insert(pos, nop)
                        pos += 1
                        i += 1
                    si.on_wait = keep
                i += 1
    return count

def _elide_redundant_updates(nc):
    """Per-instruction semaphore increments serialize on the engine's
    EVT_SEM write port (~26ns each). For any semaphore whose updates are
    all +1 increments from a single engine (so completion order == program
    order) and whose waits are all >=-immediate, only the updates that are
    the exact target of some wait are observable: a wait with threshold T
    is satisfied precisely when the T-th update lands. Keep those targets,
    drop the rest, and renumber every wait's threshold."""
    fns = nc.m.functions
    upd_by_sem = {}
    wait_refs_by_sem = {}
    for f in fns:
        for bb in f.blocks:
            for inst in bb.instructions:
                si = inst.sync_info
                if not si:
                    continue
                for u in si.on_update:
                    upd_by_sem.setdefault(u.id, []).append((inst, u))
                for w in si.on_wait:
                    wait_refs_by_sem.setdefault(w.id, []).append(w)

    n_removed = 0
    for sid, updates in upd_by_sem.items():
        waits = wait_refs_by_sem.get(sid, [])
        if not all(u.update_mode == "sem-inc" and u.update_value == 1
                   for _, u in updates):
            continue
        engines = {inst.engine for inst, _ in updates}
        if len(engines) != 1:
            continue
        if not all(w.wait_mode == "sem-ge-imm" for w in waits):
            continue
        if any(w.wait_value > len(updates) or w.wait_value < 1 for w in waits):
            continue
        # 1-based target indices that must survive; always keep the final
        # update so the value a drain might observe still advances fully.
        targets = {w.wait_value for w in waits}
        targets.add(len(updates))
        new_rank = {}
        rank = 0
        for idx, (inst, u) in enumerate(updates, start=1):
            if idx in targets:
                rank += 1
                new_rank[idx] = rank
            else:
                inst.sync_info.on_update = [
                    x for x in inst.sync_info.on_update if x is not u
                ]
                n_removed += 1
        for w in waits:
            w.wait_value = new_rank[w.wait_value]
    return n_removed


# ---------------------------------------------------------------------------
# Model constants (hardcoded per problem spec)
# ---------------------------------------------------------------------------
HID = 2048
NH = 32
NKV = 8
D = 64
B = 2
GRID = 16           # x_dim = y_dim = z_dim
BS = 8              # block size per axis
T = BS * BS * BS    # 512 tokens per block
NBLOCKS = 16        # total 3D blocks (B * 2*2*2)
N_CORES = 8
BPC = NBLOCKS // N_CORES  # blocks per core = 2
TC = BPC * T        # tokens per core = 1024
KC = HID // 128     # 16 contraction chunks

BF16 = mybir.dt.bfloat16
F32 = mybir.dt.float32

_PROGRAM = None


def _build_program():
    nc = bass.Bass("TRN2", target_bir_lowering=False, debug=False,
                   num_devices=N_CORES)

    hbT = nc.dram_tensor("hbT", [HID, TC], BF16, kind="ExternalInput")
    wqT = nc.dram_tensor("wqT", [HID, NH * D], BF16, kind="ExternalInput")
    wkT = nc.dram_tensor("wkT", [HID, NKV * D], BF16, kind="ExternalInput")
    wvT = nc.dram_tensor("wvT", [HID, NKV * D], BF16, kind="ExternalInput")
    woT = nc.dram_tensor("woT", [NH * D, HID], BF16, kind="ExternalInput")
    out = nc.dram_tensor("out", [HID, TC], F32, kind="ExternalOutput")

    QW = NH * D       # 2048
    KW = NKV * D      # 512

    with TileContext(nc) as tc:
        with tc.tile_pool(name="persist", bufs=1) as cpool:
            # kTd per group g (kv heads 2g, 2g+1): local head jl on both
            # partition halves; cols jl*TC + b*T + t
            kTdg = [cpool.tile([128, 2 * TC], BF16, tag=f"kTd{g}",
                               name=f"kTd{g}")
                    for g in range(4)]
            # v_sbg[b][sc]: [s=128, j*128 + (v_j d | ones)] per (block,
            # s-chunk); ones cols make the PV matmul emit the softmax
            # denominator on psum rows 64-127.
            v_sbg = [[cpool.tile([128, NKV * 2 * D], BF16,
                                 tag=f"vsb{b}{sc}", name=f"vsb{b}{sc}")
                      for sc in range(4)] for b in range(2)]
            # ones-fill: only the odd 64-col blocks; on DVE (idle at start),
            # per-(b,sc) so V-proj casts pipeline behind them.
            for b in range(BPC):
                for sc in range(4):
                    dst = v_sbg[b][sc][:, :].rearrange(
                        "p (j e) -> p j e", e=2 * D)[:, :, D:2 * D]
                    nc.vector.memset(dst, 1.0)

            # o^T per (group, block): [128 = pair d, pl*T + t]
            oTbg = [[cpool.tile([128, 4 * T], BF16, tag=f"oT{g}{b}",
                                name=f"oT{g}{b}")
                     for b in range(BPC)] for g in range(4)]

            with (
                tc.tile_pool(name="wo", bufs=3) as wopool,
                tc.tile_pool(name="outsb", bufs=2) as outpool,
                tc.tile_pool(name="chunks", bufs=1) as ckpool,
            ):
                # hbT + wv chunk loads interleaved on two DMA queues so
                # chunk k lands ~k*1us in; V proj (k-outer) chases them.
                # chunk 0 is split per block so the first V matmul only
                # waits on wv0 + half of hb chunk 0
                hbk0 = [ckpool.tile([128, T], BF16, tag=f"hbk0{b}",
                                    name=f"hbk0{b}") for b in range(BPC)]
                hbk = [None] + [ckpool.tile([128, TC], BF16, tag=f"hbk{k}",
                                            name=f"hbk{k}")
                                for k in range(1, KC)]

                def hb(k, b, c0, c1):
                    if k == 0:
                        return hbk0[b][:, c0:c1]
                    return hbk[k][:, T * b + c0:T * b + c1]

                def load_wk():
                    # per-kv-head-group column strips: K proj for group g
                    # only waits on its own 0.5MB strip (group 0 lands
                    # right after the hb evens), and the sync queue issues
                    # 4 DMAs instead of 16
                    ts = []
                    for g in range(4):
                        t = ckpool.tile([128, KC * 128], BF16, tag=f"wkg{g}",
                                        name=f"wkg{g}")
                        nc.sync.dma_start(
                            out=t[:, :].rearrange("p (k m) -> p k m", m=128),
                            in_=wkT[:, 128 * g:128 * (g + 1)]
                            .rearrange("(k p) m -> p k m", p=128),
                        )
                        ts.append(t)
                    return ts

                def load_wq_quarter(q):
                    # alternating tags: quarter q's DMA waits only on
                    # quarter q-2's readers, so it prefetches one group
                    # ahead and overlaps the previous group's matmuls
                    ts = []
                    for k in range(KC):
                        t = ckpool.tile([128, QW // 4], BF16,
                                        tag=f"wq{'AB'[q % 2]}{k}",
                                        name=f"wq{q}_{k}")
                        nc.sync.dma_start(
                            out=t[:, :],
                            in_=wqT[128 * k:128 * (k + 1),
                                    (QW // 4) * q:(QW // 4) * (q + 1)])
                        ts.append(t)
                    return ts

                # ---------------- V projection, k-outer -------------------
                # 8 psum banks (b, c); matmuls for chunk k start as soon as
                # hbk[k]/wvk[k] land. Last chunk's matmuls interleave with
                # their psum-drain casts so K proj isn't gated on a serial
                # cast tail (the next PSUM pools reuse these banks).
                with (
                    tc.tile_pool(name="wvp", bufs=1) as wvpool,
                    tc.tile_pool(name="ps_v", bufs=1, space="PSUM") as ps_v,
                ):
                    wvk = [wvpool.tile([128, KW], BF16, tag=f"wvk{k}",
                                       name=f"wvk{k}") for k in range(KC)]
                    for k in range(KC):
                        q = nc.sync if k % 2 == 0 else nc.gpsimd
                        q.dma_start(out=wvk[k][:, :],
                                    in_=wvT[128 * k:128 * (k + 1), :])
                        if k == 0:
                            for b in range(BPC):
                                q.dma_start(
                                    out=hbk0[b][:, :],
                                    in_=hbT[0:128, T * b:T * (b + 1)])
                        else:
                            q.dma_start(out=hbk[k][:, :],
                                        in_=hbT[128 * k:128 * (k + 1), :])
                    vps = [[ps_v.tile([128, KW], F32, tag=f"psv{b}{c}", name=f"psv{b}{c}")
                            for c in range(4)] for b in range(BPC)]
                    for k in range(KC - 2):
                        for b in range(BPC):
                            for c in range(4):
                                nc.tensor.matmul(
                                    vps[b][c][:, :],
                                    lhsT=hb(k, b, 128 * c, 128 * c + 128),
                                    rhs=wvk[k][:, :],
                                    start=(k == 0), stop=False,
                                )
                    wkk = load_wk()
                    wqk = load_wq_quarter(0)
                    # last two chunks per tile, cast immediately after each
                    # tile's stop so the 8 drain casts overlap the remaining
                    # matmuls instead of trailing them
                    for b in range(BPC):
                        for c in range(4):
                            for k in (KC - 2, KC - 1):
                                nc.tensor.matmul(
                                    vps[b][c][:, :],
                                    lhsT=hb(k, b, 128 * c, 128 * c + 128),
                                    rhs=wvk[k][:, :],
                                    start=False, stop=(k == KC - 1),
                                )
                            dst = v_sbg[b][c][:, :].rearrange(
                                "p (j e) -> p j e", e=2 * D)[:, :, 0:D]
                            srcv = vps[b][c][:, :].rearrange(
                                "p (j d) -> p j d", d=D)
                            # alternate the 8 drain casts between DVE and
                            # ACT (copy is in every act table) so the next
                            # PSUM pools aren't gated on a serial cast tail
                            if c % 2 == 0:
                                nc.vector.tensor_copy(dst, srcv)
                            else:
                                nc.scalar.activation(
                                    dst, srcv,
                                    mybir.ActivationFunctionType.Copy)

                def load_wo(mc):
                    wo = wopool.tile([128, KC * 128], BF16, tag="wo", name="wo")
                    nc.gpsimd.dma_start(
                        out=wo[:, :].rearrange("p (k m) -> p k m", m=128),
                        in_=woT[:, 128 * mc:128 * (mc + 1)]
                        .rearrange("(k p) m -> p k m", p=128),
                    )
                    return wo

                wo_tiles = []

                with (
                    tc.tile_pool(name="qTp", bufs=4) as qpool,
                    tc.tile_pool(name="pT", bufs=10) as ppool,
                    tc.tile_pool(name="lv", bufs=4) as lvpool,
                    tc.tile_pool(name="ps_proj", bufs=2, space="PSUM") as ps_proj,
                    tc.tile_pool(name="ps_st", bufs=2, space="PSUM") as ps_st,
                    tc.tile_pool(name="ps_pv", bufs=2, space="PSUM") as ps_pv,
                ):
                    def k_proj(g):
                        for b in range(BPC):
                            ps = ps_proj.tile([128, T], F32, tag="ps", name="ps")
                            for k in range(KC):
                                nc.tensor.matmul(
                                    ps[:, :],
                                    lhsT=wkk[g][:, 128 * k:128 * k + 128],
                                    rhs=hb(k, b, 0, T),
                                    start=(k == 0), stop=(k == KC - 1),
                                )
                            for jl, lo in ((0, 0), (1, 64)):
                                src = ps[lo:lo + 64, :]
                                nc.vector.tensor_copy(
                                    kTdg[g][0:64,
                                            TC * jl + T * b:TC * jl + T * (b + 1)],
                                    src)
                                nc.vector.tensor_copy(
                                    kTdg[g][64:128,
                                            TC * jl + T * b:TC * jl + T * (b + 1)],
                                    src)

                    # --- pipelined attention unit machinery ---------------
                    # One "unit" = (pair, b). QK+exp of unit i interleave
                    # with PV of unit i-1.
                    class Unit:
                        def __init__(self, g, pl, b, qTp):
                            self.g, self.pl, self.b, self.qTp = g, pl, b, qTp
                            self.pts = []     # 4 exp'd score tiles
                            self.last_exp = None

                    def emit_qk_pair(u, sci2):
                        """Emit QK unit sci2 (s-chunk) for unit u: 2
                        row-tiled concurrent MMs + exp."""
                        g, pl, b = u.g, u.pl, u.b
                        jl = pl // 2
                        st = ps_st.tile([128, 2 * T], F32, tag="st", name="st")
                        for half in range(2):
                            nc.tensor.matmul(
                                st[:, T * half:T * (half + 1)],
                                lhsT=kTdg[g][64 * half:64 * half + 64,
                                             TC * jl + T * b + 128 * sci2:
                                             TC * jl + T * b + 128 * sci2 + 128],
                                rhs=u.qTp[64 * half:64 * half + 64, :],
                                start=True, stop=True,
                            )
                        p_t = ppool.tile([128, 2 * T], BF16, tag="pT", name="pT")
                        ei = nc.scalar.activation(
                            p_t[:, :], st[:, :],
                            mybir.ActivationFunctionType.Exp,
                        )
                        u.last_exp = ei.ins
                        u.pts.append(p_t)

                    def emit_pv_half(u, half, group_lvs):
                        g, pl, b = u.g, u.pl, u.b
                        jl = pl // 2
                        po = ps_pv.tile([128, T], F32, tag="po", name="po")
                        for sc in range(4):
                            nc.tensor.matmul(
                                po[:, :],
                                lhsT=v_sbg[b][sc][:, 128 * (2 * g + jl):
                                                  128 * (2 * g + jl) + 128],
                                rhs=u.pts[sc][:, T * half:T * (half + 1)],
                                start=(sc == 0), stop=(sc == 3),
                            )
                        u.po = getattr(u, 'po', [None, None])
                        u.po[half] = po
                        if half == 1:
                            # park denominators (f32) + unnormalized o^T;
                            # psum frees immediately. Reciprocal per unit as
                            # exp(-ln(x)) on ACT: ln and exp live in ONE act
                            # table (natural_log_exp_and_others), so this
                            # costs zero table swaps and spreads evenly
                            # between the score exps instead of bulging at
                            # group boundaries like the reciprocal table
                            # round-trip did.
                            lv = lvpool.tile([128, T], F32, tag="lv",
                                             name="lv")
                            nc.vector.tensor_copy(lv[0:64, :],
                                                  u.po[0][64:128, :])
                            nc.vector.tensor_copy(lv[64:128, :],
                                                  u.po[1][64:128, :])
                            nc.vector.tensor_copy(
                                oTbg[g][b][0:64, T * (pl % 4):T * (pl % 4 + 1)],
                                u.po[0][0:64, :])
                            nc.vector.tensor_copy(
                                oTbg[g][b][64:128, T * (pl % 4):T * (pl % 4 + 1)],
                                u.po[1][0:64, :])
                            nc.scalar.activation(
                                lv[:, :], lv[:, :],
                                mybir.ActivationFunctionType.Ln)
                            nc.scalar.activation(
                                lv[:, :], lv[:, :],
                                mybir.ActivationFunctionType.Exp,
                                scale=-1.0)
                            pending_tt.append((u.g, u.pl, u.b, lv))

                    def drain_one_tt():
                        # normalize multiplies run one unit behind so the
                        # DVE never blocks waiting on this unit's ACT chain
                        if not pending_tt:
                            return
                        g, pl, b, lv = pending_tt.pop(0)
                        nc.vector.tensor_tensor(
                            out=oTbg[g][b][:, T * (pl % 4):T * (pl % 4 + 1)],
                            in0=oTbg[g][b][:, T * (pl % 4):T * (pl % 4 + 1)],
                            in1=lv[:, :],
                            op=mybir.AluOpType.mult,
                        )

                    pending = []          # units whose PV is not yet emitted
                    pending_tt = []       # units normalized but not yet multiplied

                    def drain_pending_half(_unused):
                        """Emit the next PV half of the oldest pending
                        unit."""
                        if not pending:
                            return
                        u = pending[0]
                        h = getattr(u, 'next_half', 0)
                        emit_pv_half(u, h, None)
                        if h == 1:
                            pending.pop(0)
                            drain_one_tt()
                        else:
                            u.next_half = 1

                    glvs = [[] for _ in range(4)]
                    for g in range(4):
                        k_proj(g)
                        if g < 3:
                            wqk_next = load_wq_quarter(g + 1)
                        for i, pl in enumerate(range(4)):
                            mq = 4 * g + pl
                            units = []
                            for b in range(BPC):
                                qTp = qpool.tile([128, T], BF16, tag="qTp", name="qTp")
                                ps = ps_proj.tile([128, T], F32, tag="ps", name="ps")
                                for k in range(KC):
                                    nc.tensor.matmul(
                                        ps[:, :],
                                        lhsT=wqk[k][:, 128 * pl:128 * pl + 128],
                                        rhs=hb(k, b, 0, T),
                                        start=(k == 0), stop=(k == KC - 1),
                                    )
                                nc.vector.tensor_copy(qTp[:, :], ps[:, :])
                                units.append(Unit(g, pl, b, qTp))
                            for u in units:
                                emit_qk_pair(u, 0)
                                emit_qk_pair(u, 1)
                                drain_pending_half(glvs)
                                emit_qk_pair(u, 2)
                                emit_qk_pair(u, 3)
                                drain_pending_half(glvs)
                                pending.append(u)
                        if g < 3:
                            wqk = wqk_next

                    # drain the tail of the PV pipeline and the last
                    # normalization multiplies
                    while pending:
                        drain_pending_half(glvs)
                        drain_pending_half(glvs)
                    while pending_tt:
                        drain_one_tt()

                # ------------ output projection ----------------------
                wo_tiles.extend(load_wo(mc) for mc in range(3))
                with tc.tile_pool(name="ps_wo", bufs=2, space="PSUM") as ps_wo:
                    for mc in range(KC):
                        wo = wo_tiles[mc]
                        if mc + 3 < KC:
                            wo_tiles.append(load_wo(mc + 3))
                        if mc == 0:
                            # first tile: run groups 0-2 of both blocks
                            # before touching group 3 (whose normalize may
                            # still be in flight on DVE).
                            pss = []
                            for b in range(BPC):
                                ps = ps_wo.tile([128, T], F32, tag="psf", name="psf")
                                pss.append(ps)
                                for k in range(12):
                                    nc.tensor.matmul(
                                        ps[:, :],
                                        lhsT=wo[:, 128 * k:128 * k + 128],
                                        rhs=oTbg[k // 4][b][:, T * (k % 4):
                                                            T * (k % 4 + 1)],
                                        start=(k == 0), stop=False,
                                    )
                            for b in range(BPC):
                                ps = pss[b]
                                for k in range(12, KC):
                                    nc.tensor.matmul(
                                        ps[:, :],
                                        lhsT=wo[:, 128 * k:128 * k + 128],
                                        rhs=oTbg[k // 4][b][:, T * (k % 4):
                                                            T * (k % 4 + 1)],
                                        start=False, stop=(k == KC - 1),
                                    )
                                osb = outpool.tile([128, T], F32, tag="osb", name="osb")
                                nc.vector.tensor_copy(osb[:, :], ps[:, :])
                                nc.sync.dma_start(
                                    out=out[128 * mc:128 * (mc + 1),
                                            T * b:T * (b + 1)],
                                    in_=osb[:, :],
                                )
                        else:
                            for b in range(BPC):
                                ps = ps_wo.tile([128, T], F32, tag="psf", name="psf")
                                for k in range(KC):
                                    nc.tensor.matmul(
                                        ps[:, :],
                                        lhsT=wo[:, 128 * k:128 * k + 128],
                                        rhs=oTbg[k // 4][b][:, T * (k % 4):
                                                            T * (k % 4 + 1)],
                                        start=(k == 0), stop=(k == KC - 1),
                                    )
                                osb = outpool.tile([128, T], F32, tag="osb", name="osb")
                                nc.vector.tensor_copy(osb[:, :], ps[:, :])
                                nc.sync.dma_start(
                                    out=out[128 * mc:128 * (mc + 1),
                                            T * b:T * (b + 1)],
                                    in_=osb[:, :],
                                )

    _split_excess_waits(nc)
    return nc


def _get_program():
    global _PROGRAM
    if _PROGRAM is None:
        _PROGRAM = _build_program()
    return _PROGRAM


def _to_blocks_tokens(x):
    """[B, L, F] -> [NBLOCKS, T, F] with the reference's 3D block order."""
    Bn, L, F = x.shape
    n = GRID // BS
    x = x.reshape(Bn, n, BS, n, BS, n, BS, F)
    x = x.transpose(0, 1, 3, 5, 2, 4, 6, 7)
    return x.reshape(Bn * n * n * n, BS * BS * BS, F)


def _from_blocks_tokens(x):
    """[NBLOCKS, T, F] -> [B, L, F] inverse of _to_blocks_tokens."""
    NBf, Tf, F = x.shape
    n = GRID // BS
    x = x.reshape(B, n, n, n, BS, BS, BS, F)
    x = x.transpose(0, 1, 4, 2, 5, 3, 6, 7)
    return x.reshape(B, GRID * GRID * GRID, F)


def kernel(hidden_states, Wq, Wk, Wv, Wo, x_dim, y_dim, z_dim):
    hidden_states = np.asarray(hidden_states, dtype=np.float32)
    Wq = np.asarray(Wq, dtype=np.float32)
    Wk = np.asarray(Wk, dtype=np.float32)
    Wv = np.asarray(Wv, dtype=np.float32)
    Wo = np.asarray(Wo, dtype=np.float32)

    bf = ml_dtypes.bfloat16
    scale = 1.0 / np.sqrt(D)
    wqT = np.ascontiguousarray((Wq.T * scale).astype(bf))  # [HID, 2048]
    wkT = np.ascontiguousarray(Wk.T.astype(bf))            # [HID, 512]
    wvT = np.ascontiguousarray(Wv.T.astype(bf))            # [HID, 512]
    woT = np.ascontiguousarray(Wo.T.astype(bf))            # [2048, HID]

    blocks = _to_blocks_tokens(hidden_states)              # [16, 512, HID]

    in_maps = []
    for c in range(N_CORES):
        hb = blocks[BPC * c:BPC * (c + 1)]                 # [2, 512, HID]
        hbT = np.ascontiguousarray(
            hb.transpose(2, 0, 1).reshape(HID, TC).astype(bf)
        )
        in_maps.append({
            "hbT": hbT, "wqT": wqT, "wkT": wkT, "wvT": wvT, "woT": woT,
        })

    global _LAST_IN_MAPS
    _LAST_IN_MAPS = in_maps
    nc = _get_program()
    res = run_bass_kernel_spmd(nc, in_maps, list(range(N_CORES)))

    out_blocks = np.empty((NBLOCKS, T, HID), dtype=np.float32)
    for c in range(N_CORES):
        o = res.results[c]["out"]                          # [HID, 1024]
        for b in range(BPC):
            out_blocks[BPC * c + b] = o[:, T * b:T * (b + 1)].T
    return _from_blocks_tokens(out_blocks)
